# revision 28
# baseline (speedup 1.0000x reference)
"""Trainium2 Bass kernel for nn_Camada_33612414059004.

Computes, for x:[B,N,D,S], M:[N,N], w_syn:[N,D,S], b_dend:[N,D],
w_dend:[N,D], b_soma:[N]:

    xm    = einsum('bids,oi->bods', x, M)
    dend  = tanh(einsum('bnds,nds->bnd', xm, w_syn) + b_dend)
    soma  = einsum('bnd,nd->bn', dend, w_dend) + b_soma
    out   = sigmoid(soma)                                  # [B, N]

Sharding: data-parallel over batch across 8 NeuronCores (B=64 -> 8/core),
zero cross-core communication.  Per core the dominant work is the
connectivity matmul  M[o,i] @ x[i, (b,d,s)]  in fp8-e4m3 with
perf_mode=DoubleRow (the 0/1 connectivity matrix is exact in fp8; x
quantization costs ~0.5% final rel err vs the 2e-2 gate): 4 K=256
chunk-pairs x 8 o-tiles x 2 halves = 64 matmuls of N=512 at ~242ns warm.

Each o-tile accumulates into one [128, 1024] two-bank PSUM tile.
Postprocess per tile, balanced so every engine stays under the ~2.0us
PE pace:
  Scalar: one ACT-copy PSUM -> bf16 [128,1024] ((1024+352)/1.2), tanh,
          sigmoid(+b_soma per-partition bias).
  Vector: one bf16 2x w_syn multiply, s-reduce as bf16 pairwise tree
          (16->8->4 adds, then a 4->1 tensor_reduce), b_dend add on
          even tiles.
  GpSimd: b_dend add on odd tiles, soma stage (w_dend mult + d-tree),
          mt/params DMA issue.
The last tile skips the scalar copy and runs its multiplies directly
from PSUM on DVE (h0 early via h-outer matmul order) with the soma on
DVE, minimizing the serial tail after the final matmul.

Schedule: o-tiles 0-1 accumulate k-outer, riding the input DMA stream;
tiles 2-7 run k-inner so each tile's postprocess overlaps the next
tile's matmuls.  x chunk-pairs on the Sync HWDGE; the first mt
chunk-pair on Scalar (free early), the rest + params on GpSimd; the
first x/mt chunk-pair is split in column halves so the first real
matmul waits on only 2x128KB.
"""

import numpy as np
import ml_dtypes
from contextlib import ExitStack

import concourse.bass as bass
import concourse.mybir as mybir
import concourse.tile as tile

B, N, D, S = 64, 1024, 8, 16
NCORES = 8
BC = B // NCORES          # batches per core = 8
DS = D * S                # 128
P = 128                   # SBUF partitions
KT = N // P               # 8 contraction chunks (input neurons)
KT2 = KT // 2             # 4 DoubleRow chunk pairs (K=256 each)
OT = N // P               # 8 output-neuron tiles
FH = 512                  # one fp32 PSUM bank
BD = BC * D               # 64
GRP = 2                   # o-tiles in the k-outer leading group

F32 = mybir.dt.float32
BF16 = mybir.dt.bfloat16
FP8 = mybir.dt.float8e4

# packed fp32 params: b_dend | b_soma (all o-tile-major)
PF_BD, PF_BS = 0, OT * D
PF_COLS = OT * D + OT                    # 72
# packed bf16 params: w_syn | w_dend
PB_WS, PB_WD = 0, OT * DS
PB_COLS = OT * DS + OT * D               # 1088

_NC_CACHE = {}


def legalize_waits(nc, max_attached=1):
    """Split multi-semaphore waits onto preceding same-engine NOPs.

    The walrus build in this environment accepts at most one sync-wait
    command per instruction (setupSyncWait: "Too many sync wait commands"),
    but Tile attaches one wait per out-of-date engine clock.  An engine is
    in-order, so hoisting the extra waits onto NOPs immediately before the
    instruction is semantics-preserving.
    """
    nid = 0
    for f in nc.m.functions:
        for blk in f.blocks:
            new = []
            changed = False
            for inst in blk.instructions:
                si = inst.sync_info
                if si is not None and si.on_wait and len(si.on_wait) > max_attached:
                    waits = list(si.on_wait)
                    for w in waits[:-max_attached]:
                        nid += 1
                        nop = mybir.InstNoOp(name=f"WSPLIT-{nid}", ins=[], outs=[])
                        nop.engine = inst.engine
                        nop.sync_info = mybir.SyncInfo(on_wait=[w], on_update=[])
                        new.append(nop)
                    inst.sync_info = mybir.SyncInfo(
                        on_wait=waits[-max_attached:], on_update=list(si.on_update)
                    )
                    changed = True
                new.append(inst)
            if changed:
                blk.instructions = new
    return nc


def build_nc(legalize=True):
    """Build the single-core Bass program (SPMD: same program on all cores)."""
    nc = bass.Bass()
    mt = nc.declare_dram_parameter("mt", [N, N], FP8, isOutput=False)
    xc = nc.declare_dram_parameter("xc", [N, BC * DS], FP8, isOutput=False)
    pf32 = nc.declare_dram_parameter("pf32", [P, PF_COLS], F32, isOutput=False)
    pbf = nc.declare_dram_parameter("pbf", [P, PB_COLS], BF16, isOutput=False)
    out = nc.declare_dram_parameter("out", [P, OT * BC], F32, isOutput=True)

    AF = mybir.ActivationFunctionType
    AX = mybir.AxisListType
    OP = mybir.AluOpType
    DR = mybir.MatmulPerfMode.DoubleRow

    with tile.TileContext(nc) as tc, ExitStack() as ctx:
        wpool = ctx.enter_context(tc.tile_pool(name="weights", bufs=1))
        xpool = ctx.enter_context(tc.tile_pool(name="xin", bufs=1))
        pspool = ctx.enter_context(tc.tile_pool(name="ps", bufs=4, space="PSUM"))
        prpool = ctx.enter_context(tc.tile_pool(name="prp", bufs=3))
        smpool = ctx.enter_context(tc.tile_pool(name="smp", bufs=3))

        # --- PE pre-warm: dummy matmuls on memset scratch while the first
        # input chunk is in flight, lifting the HAM clock gate (1.2 ->
        # 2.4 GHz needs ~3.4us of sustained PE activity).  The memset runs
        # on Vector (idle early) so GpSimd can start issuing DMAs. ---
        warm_sb = wpool.tile([P, FH], BF16, tag="warm", name="warm_sb")
        nc.vector.memset(warm_sb[:], 0.0)
        warm_ps = pspool.tile([P, 2 * FH], F32, tag="ps", name="warm_ps")
        for _ in range(12):
            nc.tensor.matmul(
                warm_ps[:, 0:FH], lhsT=warm_sb[:, 0:P], rhs=warm_sb[:],
                start=True, stop=True,
            )

        # --- input DMAs.  x: per chunk-pair [128, 2048] tiles, rows
        # host-interleaved (kp, p, g) so each partition's 2 KB is one
        # contiguous DRAM run.  mt: host-repacked per o-tile so tile t's
        # weights for all chunk-pairs are one [128, 1024] contiguous
        # block.  The two HWDGE queues (Sync, Scalar) share ~270 GB/s, so
        # the interleave below lands mtt0/mtt1 + all of x first (the
        # postprocess of every tile is gated on x completing), then the
        # later tiles' mt blocks.  Params ride the GpSimd SWDGE. ---
        x_tiles = [None] * KT2
        mtt_tiles = [None] * OT

        def xdma(eng, k):
            xt = xpool.tile([P, 2 * BC * DS], FP8, tag=f"x{k}", name=f"x{k}")
            eng.dma_start(
                xt[:],
                xc[k * 2 * P:(k + 1) * 2 * P, :].rearrange(
                    "(p g) c -> p (g c)", g=2),
            )
            x_tiles[k] = xt

        def mdma(eng, t):
            mtk = xpool.tile([P, KT2 * 2 * P], FP8, tag=f"mtt{t}", name=f"mtt{t}")
            eng.dma_start(mtk[:], mt[t * P:(t + 1) * P, :])
            mtt_tiles[t] = mtk

        mdma(nc.sync, 0)
        mdma(nc.scalar, 1)
        xdma(nc.sync, 0)
        xdma(nc.scalar, 1)
        xdma(nc.sync, 2)
        xdma(nc.scalar, 3)
        mdma(nc.sync, 2)
        mdma(nc.scalar, 3)
        mdma(nc.sync, 4)
        mdma(nc.scalar, 5)
        mdma(nc.sync, 6)
        mdma(nc.scalar, 7)

        pbf_sb = wpool.tile([P, PB_COLS], BF16, tag="pbf", name="pbf_sb")
        nc.gpsimd.dma_start(pbf_sb[:], pbf[:, :])
        pf_sb = wpool.tile([P, PF_COLS], F32, tag="pf32", name="pf_sb")
        nc.gpsimd.dma_start(pf_sb[:], pf32[:, :])

        out_sb = wpool.tile([P, OT * BC], F32, tag="out", name="out_sb")

        # Pre-expanded (broadcast-over-b) parameter tiles, built once on
        # the otherwise-idle GpSimd so the per-tile bias/soma ops become
        # flat 2D/3D patterns (4D broadcast APs dispatch slowly there).
        bdend_x = wpool.tile([P, OT * BD], F32, tag="bdx", name="bdend_x")
        nc.gpsimd.tensor_copy(
            bdend_x[:].rearrange("p (t b d) -> p t b d", t=OT, d=D),
            pf_sb[:, PF_BD:PF_BD + OT * D]
            .rearrange("p (t d) -> p t d", t=OT).unsqueeze(2)
            .broadcast_to([P, OT, BC, D]),
        )
        wdend_x = wpool.tile([P, OT * BD], BF16, tag="wdx", name="wdend_x")
        nc.gpsimd.tensor_copy(
            wdend_x[:].rearrange("p (t b d) -> p t b d", t=OT, d=D),
            pbf_sb[:, PB_WD:PB_WD + OT * D]
            .rearrange("p (t d) -> p t d", t=OT).unsqueeze(2)
            .broadcast_to([P, OT, BC, D]),
        )
        bsoma_x = wpool.tile([P, OT * BC], F32, tag="bsx", name="bsoma_x")
        nc.gpsimd.tensor_copy(
            bsoma_x[:].rearrange("p (t b) -> p t b", t=OT),
            pf_sb[:, PF_BS:PF_BS + OT].unsqueeze(2)
            .broadcast_to([P, OT, BC]),
        )

        def wsyn_bf(t):
            return (pbf_sb[:, PB_WS + t * DS:PB_WS + (t + 1) * DS]
                    .unsqueeze(1).broadcast_to([P, BC, DS]))

        # Postprocess stages, emitted separately so each in-order engine
        # queue sees work in data-readiness order (keeping e.g. the next
        # pair's scalar copies AHEAD of the previous pair's tanh).
        def pp_front(te, ps_a, ps_b):
            # Scalar: two PSUM -> SBUF bf16 copies into one [128, 2048] tile.
            cp = prpool.tile([P, 4 * FH], BF16, tag="cp", name=f"cp{te}")
            nc.scalar.activation(cp[:, 0:2 * FH], ps_a[:], AF.Copy)
            nc.scalar.activation(cp[:, 2 * FH:4 * FH], ps_b[:], AF.Copy)
            return cp

        def pp_mid_pair(te, cp):
            # DVE: one bf16 2x w_syn multiply + the s-tree (16->8->4 adds,
            # 4->1 reduce).  Same-engine, in-order chain.
            prod = prpool.tile([P, 4 * FH], BF16, tag="prod", name=f"prod{te}")
            wsyn2 = (pbf_sb[:, PB_WS + te * DS:PB_WS + (te + 2) * DS]
                     .rearrange("p (u q) -> p u q", u=2).unsqueeze(2)
                     .broadcast_to([P, 2, BC, DS]))
            nc.vector.tensor_mul(
                prod[:].rearrange("p (u b q) -> p u b q", u=2, b=BC),
                cp[:].rearrange("p (u b q) -> p u b q", u=2, b=BC),
                wsyn2,
            )
            return tree(te, prod, 2 * BD)

        def tree(tag, prod, n):
            pv = prod[:].rearrange("p (bd s) -> p bd s", s=S)
            gr1 = smpool.tile([P, n * 8], BF16, tag="gr1", name=f"gr1{tag}")
            g1 = gr1[:].rearrange("p (bd s) -> p bd s", s=8)
            nc.vector.tensor_add(g1, pv[:, :, 0:8], pv[:, :, 8:16])
            gr2 = smpool.tile([P, n * 4], BF16, tag="gr2", name=f"gr2{tag}")
            g2 = gr2[:].rearrange("p (bd s) -> p bd s", s=4)
            nc.vector.tensor_add(g2, g1[:, :, 0:4], g1[:, :, 4:8])
            dp = smpool.tile([P, n], F32, tag="dp", name=f"dp{tag}")
            nc.vector.tensor_reduce(dp[:], g2, axis=AX.X, op=OP.add)
            return dp

        def pp_tail(te, dp, n, gps, sig_bias=None):
            # bias add + w_dend soma (GpSimd or DVE), tanh/sigmoid on
            # Scalar.  n = BD for singles, 2*BD for pairs.
            nt = n // BD
            eng = nc.gpsimd if gps else nc.vector
            eng.tensor_add(dp[:], dp[:], bdend_x[:, te * BD:te * BD + n])
            dend = smpool.tile([P, n], BF16, tag="dend", name=f"dend{te}")
            nc.scalar.activation(dend[:], dp[:], AF.Tanh)
            sp = smpool.tile([P, n], BF16, tag="sp", name=f"sp{te}")
            eng.tensor_mul(sp[:], dend[:], wdend_x[:, te * BD:te * BD + n])
            soma = smpool.tile([P, nt * BC], F32, tag="soma", name=f"soma{te}")
            spv = sp[:].rearrange("p (ub d) -> p ub d", d=D)
            if gps:
                r1 = smpool.tile([P, nt * BC * 4], BF16, tag="r1", name=f"r1{te}")
                r1v = r1[:].rearrange("p (ub d) -> p ub d", d=4)
                nc.gpsimd.tensor_add(r1v, spv[:, :, 0:4], spv[:, :, 4:8])
                r2 = smpool.tile([P, nt * BC * 2], BF16, tag="r2", name=f"r2{te}")
                r2v = r2[:].rearrange("p (ub d) -> p ub d", d=2)
                nc.gpsimd.tensor_add(r2v, r1v[:, :, 0:2], r1v[:, :, 2:4])
                sm0 = smpool.tile([P, nt * BC], F32, tag="sm0", name=f"sm0{te}")
                nc.gpsimd.tensor_add(
                    sm0[:].unsqueeze(2), r2v[:, :, 0:1], r2v[:, :, 1:2])
                nc.gpsimd.tensor_add(
                    soma[:], sm0[:], bsoma_x[:, te * BC:te * BC + nt * BC])
                nc.scalar.activation(
                    out_sb[:, te * BC:te * BC + nt * BC], soma[:], AF.Sigmoid)
            else:
                nc.vector.tensor_reduce(soma[:], spv, axis=AX.X, op=OP.add)
                nc.scalar.activation(
                    out_sb[:, te * BC:te * BC + nt * BC], soma[:], AF.Sigmoid,
                    bias=sig_bias,
                )

        def pp_mid_single(t, pst):
            # Tail-latency variant: DVE multiplies straight from PSUM.
            prod = prpool.tile([P, BC * DS], BF16, tag="prods", name=f"prod{t}")
            nc.vector.tensor_mul(
                prod[:].rearrange("p (b q) -> p b q", b=BC),
                pst[:].rearrange("p (b q) -> p b q", b=BC),
                wsyn_bf(t),
            )
            return tree(f"s{t}", prod, BD)

        def mm(pst, t, k, h):
            rhs = (x_tiles[k][:].rearrange("p (g c) -> p g c", g=2)
                   [:, :, h * FH:(h + 1) * FH])
            lhs = (mtt_tiles[t][:].rearrange("p (kp g c) -> p kp g c",
                                             kp=KT2, g=2)[:, k, :, :])
            nc.tensor.matmul(
                pst[:, h * FH:(h + 1) * FH], lhsT=lhs, rhs=rhs,
                start=(k == 0), stop=(k == KT2 - 1), perf_mode=DR,
            )

        # Schedule: lead pair {0,1} k-outer rides the x DMA stream; pairs
        # {2,3}, {4,5} and singles 6, 7 k-inner.  Stage emission is
        # software-pipelined: each pair's scalar copies are enqueued ahead
        # of the PREVIOUS pair's tanh/sigmoid so the in-order Scalar queue
        # never head-of-line blocks the DVE stream.
        pst = {}
        for t in range(GRP):
            pst[t] = pspool.tile([P, 2 * FH], F32, tag="ps", name=f"ps{t}")
        for k in range(KT2):
            for t in range(GRP):
                for h in range(2):
                    mm(pst[t], t, k, h)

        def mms(t):
            pst[t] = pspool.tile([P, 2 * FH], F32, tag="ps", name=f"ps{t}")
            for k in range(KT2):
                for h in range(2):
                    mm(pst[t], t, k, h)

        cp01 = pp_front(0, pst[0], pst[1])
        mms(2)
        mms(3)
        cp23 = pp_front(2, pst[2], pst[3])
        dp01 = pp_mid_pair(0, cp01)
        pp_tail(0, dp01, 2 * BD, gps=True)
        mms(4)
        mms(5)
        cp45 = pp_front(4, pst[4], pst[5])
        dp23 = pp_mid_pair(2, cp23)
        pp_tail(2, dp23, 2 * BD, gps=True)
        mms(6)
        dp45 = pp_mid_pair(4, cp45)
        pp_tail(4, dp45, 2 * BD, gps=True)
        mms(7)
        dp6 = pp_mid_single(6, pst[6])
        pp_tail(6, dp6, BD, gps=True)
        nc.sync.dma_start(out[:, 0:6 * BC], out_sb[:, 0:6 * BC])
        dp7 = pp_mid_single(7, pst[7])
        pp_tail(7, dp7, BD, gps=False,
                sig_bias=pf_sb[:, PF_BS + 7:PF_BS + 8])
        nc.sync.dma_start(out[:, 6 * BC:], out_sb[:, 6 * BC:])

    if legalize:
        legalize_waits(nc)
    return nc


def get_nc():
    if "nc" not in _NC_CACHE:
        _NC_CACHE["nc"] = build_nc()
    return _NC_CACHE["nc"]


def pack_params(w_syn, b_dend, w_dend, b_soma):
    """Pack per-neuron parameters into the fp32 and bf16 SBUF layouts
    (each section o-tile-major: column block t holds o-tile t's rows)."""
    ws = np.asarray(w_syn, np.float32).reshape(OT, P, DS).transpose(1, 0, 2).reshape(P, OT * DS)
    bd = np.asarray(b_dend, np.float32).reshape(OT, P, D).transpose(1, 0, 2).reshape(P, OT * D)
    wd = np.asarray(w_dend, np.float32).reshape(OT, P, D).transpose(1, 0, 2).reshape(P, OT * D)
    bs = np.asarray(b_soma, np.float32).reshape(OT, P).T
    pf = np.ascontiguousarray(np.concatenate([bd, bs], axis=1))
    pb = np.ascontiguousarray(
        np.concatenate([ws, wd], axis=1).astype(ml_dtypes.bfloat16))
    return pf, pb


def interleave_rows(a):
    """Reorder [N, C] rows from (kp, g, p) to (kp, p, g) so each SBUF
    partition's DoubleRow pair is one contiguous 2C-byte DRAM run."""
    return np.ascontiguousarray(
        a.reshape(KT2, 2, P, a.shape[1]).transpose(0, 2, 1, 3)
        .reshape(N, a.shape[1]))


def prepare_in_maps(x, matriz_conexao, w_syn, b_dend, w_dend, b_soma):
    x = np.asarray(x, dtype=np.float32)
    # mt repacked per o-tile: row (t, p), cols (kp, g, c) so tile t's
    # DoubleRow weights for all chunk-pairs are one contiguous block.
    mtT = np.asarray(matriz_conexao, np.float32).T.astype(ml_dtypes.float8_e4m3)
    mt_np = np.ascontiguousarray(
        mtT.reshape(KT2, 2, P, OT, P).transpose(3, 2, 0, 1, 4).reshape(N, N))
    pf, pb = pack_params(w_syn, b_dend, w_dend, b_soma)
    xt = np.ascontiguousarray(x.transpose(1, 0, 2, 3).reshape(N, B, DS))
    in_maps = []
    for c in range(NCORES):
        xc_np = interleave_rows(
            xt[:, c * BC:(c + 1) * BC, :].reshape(N, BC * DS)
            .astype(ml_dtypes.float8_e4m3))
        in_maps.append({"mt": mt_np, "xc": xc_np, "pf32": pf, "pbf": pb})
    return in_maps


def assemble_output(results):
    outs = []
    for c in range(NCORES):
        oc = np.asarray(results[c]["out"])          # [P, OT*BC] = (oi, (t, b))
        outs.append(oc.reshape(P, OT, BC).transpose(2, 1, 0).reshape(BC, N))
    return np.ascontiguousarray(np.concatenate(outs, axis=0).astype(np.float32))


def kernel(x, matriz_conexao, w_syn, b_dend, w_dend, b_soma):
    from concourse.bass_utils import run_bass_kernel_spmd
    in_maps = prepare_in_maps(x, matriz_conexao, w_syn, b_dend, w_dend, b_soma)
    nc = get_nc()
    res = run_bass_kernel_spmd(nc, in_maps, list(range(NCORES)))
    return assemble_output(res.results)
